# revision 1
# baseline (speedup 1.0000x reference)
"""H2GCNConv on 8 Trainium2 NeuronCores.

out = concat([A1 @ x, A2 @ x], axis=1) where A_h is sparse [N, N] given as
(row=dest, col=src, val) edge lists.

Strategy (dest-sharded SpMM via gather + segment-matmul):
  - Destination rows are partitioned across 8 cores (6250 rows each).
  - Host sorts each core's edges by destination tile (128 dest rows per
    tile), splits by column half (int16 gather-index limit), and pads each
    (tile, hop, half) section to a whole number of 128-edge chunks.
  - x is cast to fp16 and replicated; each core gathers its edges' source
    rows (512B each) straight from HBM with SWDGE dma_gather.
  - For each 128-edge chunk, a selection matrix S[e, d] = val[e] *
    (dest_local[e] == d) is built on the vector engine from a host-supplied
    iota ramp, then the tensor engine computes psum[d, :] += S.T @ msgs
    (fp16 x fp16 -> fp32 PSUM), which performs scale + segment-sum in one
    matmul. PSUM accumulates over a tile's chunks; the result is copied
    out via the scalar engine and DMA'd to the output.
"""

import sys

if "/opt/trn_rl_repo" not in sys.path:
    sys.path.insert(0, "/opt/trn_rl_repo")

import numpy as np

P = 128


def _preprocess(rows, cols, vals, n_nodes, rpc, split, ncores):
    """Shard one hop's edges by dest core/tile, sort, and compute padded
    layout. Returns (caps[t, half] in chunks, per-core padded arrays)."""
    T = -(-rpc // P)  # tiles per core
    core = rows // rpc
    local = rows - core * rpc
    t = local >> 7
    ld = local & (P - 1)
    half = (cols >= split).astype(np.int64)
    idx = (cols - half * split).astype(np.int16)

    nsec = ncores * T * 2
    key = (core * T + t) * 2 + half
    counts = np.bincount(key, minlength=nsec).reshape(ncores, T, 2)
    # capacity per (t, half): max over cores, in 128-edge chunks
    caps = -(-counts.max(axis=0) // P)  # [T, 2]
    caps = np.maximum(caps, 1)

    order = np.argsort(key, kind="stable")
    key_s = key[order]
    cs = np.concatenate([[0], np.cumsum(counts.reshape(-1))])
    rank = np.arange(len(rows)) - cs[key_s]
    return caps, counts, order, key_s, rank, idx, ld, vals


def _host_build(x, row1, col1, val1, row2, col2, val2, ncores):
    n_nodes, d_feat = x.shape
    rpc = n_nodes // ncores
    T = -(-rpc // P)
    split = -(-n_nodes // 2)
    # keep both halves within int16 gather-index range
    assert split <= 32767 and n_nodes - split <= 32767

    caps1, counts1, order1, key1_s, rank1, idx1, ld1, v1 = _preprocess(
        np.asarray(row1), np.asarray(col1), np.asarray(val1), n_nodes, rpc, split, ncores
    )
    caps2, counts2, order2, key2_s, rank2, idx2, ld2, v2 = _preprocess(
        np.asarray(row2), np.asarray(col2), np.asarray(val2), n_nodes, rpc, split, ncores
    )

    # per-core padded edge-space layout, in device order:
    #   for t in range(T): [h1 lo, h1 hi, h2 lo, h2 hi]
    sec1 = caps1.sum(axis=1)  # chunks per (t, hop1)
    sec2 = caps2.sum(axis=1)
    tile_chunks = sec1 + sec2
    tile_off_chunks = np.concatenate([[0], np.cumsum(tile_chunks)])
    tot_chunks = int(tile_off_chunks[-1])
    pad_e = tot_chunks * P

    # edge offset (within a core's padded space) for each (t, hop, half)
    off = np.zeros((2, T, 2), dtype=np.int64)
    for t in range(T):
        base = tile_off_chunks[t] * P
        off[0, t, 0] = base
        off[0, t, 1] = base + caps1[t, 0] * P
        off[1, t, 0] = base + sec1[t] * P
        off[1, t, 1] = base + sec1[t] * P + caps2[t, 0] * P

    pad_idx = np.zeros((ncores, pad_e), dtype=np.int16)
    pad_ld = np.zeros((ncores, pad_e), dtype=np.float16)
    pad_val = np.zeros((ncores, pad_e), dtype=np.float16)

    for h, (caps, order, key_s, rank, idx, ld, v) in enumerate(
        (
            (caps1, order1, key1_s, rank1, idx1, ld1, v1),
            (caps2, order2, key2_s, rank2, idx2, ld2, v2),
        )
    ):
        core_s = key_s // (T * 2)
        t_s = (key_s // 2) % T
        half_s = key_s % 2
        pos = off[h, t_s, half_s] + rank
        pad_idx[core_s, pos] = idx[order]
        pad_ld[core_s, pos] = ld[order].astype(np.float16)
        pad_val[core_s, pos] = np.asarray(v, dtype=np.float16)[order]

    # device-layout arrays
    dest_arr = np.ascontiguousarray(
        pad_ld.reshape(ncores, tot_chunks, P).transpose(0, 2, 1)
    )  # [ncores, 128, tot_chunks]
    val_arr = np.ascontiguousarray(
        pad_val.reshape(ncores, tot_chunks, P).transpose(0, 2, 1)
    )

    # idx dram layout: per section, [16, n/16] wrap replicated to 128 rows
    idx_cols = pad_e // 16
    idx_arr = np.zeros((ncores, 16, idx_cols), dtype=np.int16)
    # sections are contiguous in padded space; their wrap is independent
    sec_bounds = []
    for t in range(T):
        for h in range(2):
            for half in range(2):
                caps = caps1 if h == 0 else caps2
                n = int(caps[t, half]) * P
                o = int(off[h, t, half])
                sec_bounds.append((o, n))
    for c in range(ncores):
        for o, n in sec_bounds:
            seg = pad_idx[c, o : o + n].reshape(n // 16, 16).T
            idx_arr[c, :, o // 16 : (o + n) // 16] = seg
    idx_arr = np.tile(idx_arr, (1, 8, 1))  # [ncores, 128, idx_cols]

    maxsec = int(max(sec1.max(), sec2.max()))
    iota = np.ascontiguousarray(
        np.broadcast_to(
            np.arange(P, dtype=np.float16)[None, None, :], (P, maxsec, P)
        )
    )  # [128, maxsec, 128]

    x16 = np.asarray(x, dtype=np.float16)

    meta = dict(
        ncores=ncores,
        rpc=rpc,
        T=T,
        split=split,
        n_nodes=n_nodes,
        d_feat=d_feat,
        caps1=caps1,
        caps2=caps2,
        sec1=sec1,
        sec2=sec2,
        tile_off_chunks=tile_off_chunks,
        tot_chunks=tot_chunks,
        off=off,
        maxsec=maxsec,
        idx_cols=idx_cols,
    )
    per_core = [
        dict(
            x16=x16,
            idx=idx_arr[c],
            dest=dest_arr[c],
            val=val_arr[c],
            iota=iota,
        )
        for c in range(ncores)
    ]
    return meta, per_core


def _build_program(meta, max_tiles=None, max_hops=2):
    from concourse import bacc, mybir, tile

    T = meta["T"] if max_tiles is None else min(meta["T"], max_tiles)
    rpc = meta["rpc"]
    split = meta["split"]
    n_nodes = meta["n_nodes"]
    d = meta["d_feat"]
    caps = (meta["caps1"], meta["caps2"])
    secs = (meta["sec1"], meta["sec2"])
    off = meta["off"]
    tot_chunks = meta["tot_chunks"]
    maxsec = meta["maxsec"]
    idx_cols = meta["idx_cols"]

    nc = bacc.Bacc("TRN2", target_bir_lowering=False, debug=False,
                   num_devices=meta["ncores"])

    x16 = nc.dram_tensor("x16", [n_nodes, d], mybir.dt.float16, kind="ExternalInput")
    idx_d = nc.dram_tensor("idx", [P, idx_cols], mybir.dt.int16, kind="ExternalInput")
    dest_d = nc.dram_tensor("dest", [P, tot_chunks], mybir.dt.float16, kind="ExternalInput")
    val_d = nc.dram_tensor("val", [P, tot_chunks], mybir.dt.float16, kind="ExternalInput")
    iota_d = nc.dram_tensor("iota", [P, maxsec, P], mybir.dt.float16, kind="ExternalInput")
    out_d = nc.dram_tensor("out", [rpc, 2 * d], mybir.dt.float32, kind="ExternalOutput")

    fp16 = mybir.dt.float16
    f32 = mybir.dt.float32
    eq = mybir.AluOpType.is_equal
    mult = mybir.AluOpType.mult

    with tile.TileContext(nc) as tc:
        with (
            tc.tile_pool(name="const", bufs=1) as constp,
            tc.tile_pool(name="idx", bufs=4) as idxp,
            tc.tile_pool(name="msgs", bufs=3) as msgsp,
            tc.tile_pool(name="sel", bufs=3) as selp,
            tc.tile_pool(name="psum", bufs=4, space="PSUM") as psump,
            tc.tile_pool(name="stage", bufs=4) as stagep,
        ):
            iota_sb = constp.tile([P, maxsec, P], fp16, tag="iota")
            nc.sync.dma_start(iota_sb[:, :, :], iota_d[:, :, :])
            dest_sb = constp.tile([P, tot_chunks], fp16, tag="dest")
            nc.sync.dma_start(dest_sb[:, :], dest_d[:, :])
            val_sb = constp.tile([P, tot_chunks], fp16, tag="val")
            nc.sync.dma_start(val_sb[:, :], val_d[:, :])

            for t in range(T):
                rows = min(P, rpc - t * P)
                for h in range(max_hops):
                    sec = int(secs[h][t])
                    c0 = int(meta["tile_off_chunks"][t] + (secs[0][t] if h else 0))
                    msgs = msgsp.tile([P, sec, d], fp16, tag=f"msgs{h}")
                    for half in range(2):
                        cap = int(caps[h][t, half])
                        o = int(off[h, t, half])
                        n = cap * P
                        it = idxp.tile([P, n // 16], mybir.dt.int16, tag="idx")
                        nc.sync.dma_start(
                            it[:, :], idx_d[:, o // 16 : (o + n) // 16]
                        )
                        src = x16[0:split, :] if half == 0 else x16[split:n_nodes, :]
                        coff = (o - off[h, t, 0]) // P
                        nc.gpsimd.dma_gather(
                            msgs[:, coff : coff + cap, :],
                            src,
                            it[:, :],
                            n,
                            n,
                            d,
                            single_packet=False,
                        )
                    sel = selp.tile([P, sec, P], fp16, tag=f"sel{h}")
                    dview = dest_sb[:, c0 : c0 + sec, None].to_broadcast([P, sec, P])
                    vview = val_sb[:, c0 : c0 + sec, None].to_broadcast([P, sec, P])
                    nc.vector.tensor_tensor(
                        out=sel[:, :, :], in0=iota_sb[:, :sec, :], in1=dview, op=eq
                    )
                    nc.vector.tensor_tensor(
                        out=sel[:, :, :], in0=sel[:, :, :], in1=vview, op=mult
                    )
                    ps = psump.tile([P, d], f32, tag="ps")
                    for j in range(sec):
                        nc.tensor.matmul(
                            ps[:, :],
                            sel[:, j, :],
                            msgs[:, j, :],
                            start=(j == 0),
                            stop=(j == sec - 1),
                        )
                    st = stagep.tile([P, d], f32, tag="st")
                    nc.scalar.copy(st[:, :], ps[:, :])
                    nc.sync.dma_start(
                        out_d[t * P : t * P + rows, h * d : (h + 1) * d],
                        st[:rows, :],
                    )
    nc.compile()
    return nc


def kernel(x, row1, col1, val1, row2, col2, val2):
    from concourse.bass_utils import run_bass_kernel_spmd

    ncores = 8
    meta, per_core = _host_build(x, row1, col1, val1, row2, col2, val2, ncores)
    nc = _build_program(meta)
    res = run_bass_kernel_spmd(nc, per_core, list(range(ncores)))
    rpc = meta["rpc"]
    d = meta["d_feat"]
    out = np.empty((x.shape[0], 2 * d), dtype=np.float32)
    for c in range(ncores):
        out[c * rpc : (c + 1) * rpc] = res.results[c]["out"]
    return out



# revision 22
# speedup vs baseline: 94.1694x; 94.1694x over previous
"""H2GCNConv on 8 Trainium2 NeuronCores.

out = concat([A1 @ x, A2 @ x], axis=1) where A_h is sparse [N, N] given as
(row=dest, col=src, val) edge lists.

Strategy (dest-sharded SpMM via SWDGE gather + segment-matmul):
  - Destination-row tiles (128 rows) are distributed across the 8 cores
    with per-slot load balancing (slot s takes tiles ranked [8s, 8s+8) by
    edge count), and the host un-permutes the output rows at the end.
  - Host sorts each core's edges by (dest tile, column half, hop), pads
    each (tile, half, hop) section to whole 128-edge chunks.  The column
    half split keeps gather indices within int16 range.
  - x is cast to fp8 (e3m4, 4 mantissa bits) and replicated; each core
    runs ONE dma_gather per (tile, half) section (both hops' edges at
    once, 256B per edge) from HBM.  Gathers round-robin across 4 SWDGE
    queues so descriptor generation runs on all four Q7 core-pairs
    concurrently — the Q7 descriptor loop (~8ns/edge/pair) is the
    kernel's critical path; fp8 halves the DMA drain so it stays hidden.
  - Per (tile, half) section, a selection matrix S[e, c, d] = val[e,c] *
    (d == dest_local[e,c]) is built on the vector engine in two section-
    level passes (is_equal, then mult).  The broadcast operands (dest,
    val) are pair-duplicated on host and read with an innermost stride-1
    pair, which keeps the DVE in its 2-element/cycle mode (a plain
    stride-0 broadcast halves DVE throughput).
  - Per 128-edge chunk, the tensor engine computes
    psum[d, h*256:(h+1)*256] += S_chunk.T @ msgs_chunk as a mixed-dtype
    matmul (fp16 stationary sel x fp8e3 moving msgs -> fp32 PSUM), so
    edge weights keep fp16 precision while messages ride in fp8.  Both
    hops of a tile accumulate into a single [128, 512] PSUM bank
    (start=True zeroes the whole 2KB region).
  - PSUM is copied out through the scalar engine as fp16 and DMA'd to a
    fp16 output, which the host upcasts to fp32.  End-to-end relative
    error vs the fp64 oracle is ~1.4e-2 (dominated by fp8 message
    quantization), within the 2e-2 gate.
"""

import sys

if "/opt/trn_rl_repo" not in sys.path:
    sys.path.insert(0, "/opt/trn_rl_repo")

import numpy as np

P = 128
NQ = 4  # SWDGE queues


def _preprocess(rows, cols, vals, n_nodes, core_of_tile, slot_of_tile, T,
                split, ncores):
    """Sort one hop's edges by (core, slot, half); return per-(slot, half)
    capacities (in 128-edge chunks) and the sorted scatter metadata."""
    g = rows >> 7  # global dest tile
    core = core_of_tile[g]
    t = slot_of_tile[g]
    half = (cols >= split).astype(np.int64)
    idx = (cols - half * split).astype(np.int16)
    ld = rows & (P - 1)

    nsec = ncores * T * 2
    key = (core * T + t) * 2 + half
    counts = np.bincount(key, minlength=nsec).reshape(ncores, T, 2)
    caps = -(-counts.max(axis=0) // P)  # [T, 2] chunks
    caps = np.maximum(caps, 1)

    order = np.argsort(key, kind="stable")
    key_s = key[order]
    cs = np.concatenate([[0], np.cumsum(counts.reshape(-1))])
    rank = np.arange(len(rows)) - cs[key_s]
    return caps, order, key_s, rank, idx, ld, vals


def _host_build(x, row1, col1, val1, row2, col2, val2, ncores):
    n_nodes, d_feat = x.shape
    G = -(-n_nodes // P)       # global dest tiles
    T = -(-G // ncores)        # slots per core
    GP = T * ncores            # padded with dummy (empty) tiles
    split = -(-n_nodes // 2)
    assert split <= 32767 and n_nodes - split <= 32767

    row1 = np.asarray(row1)
    row2 = np.asarray(row2)
    # balance per-slot edge counts: slot s on each core takes one of the 8
    # tiles ranked [8s, 8s+8) by total edge count, so the per-slot cap
    # (max over cores) stays close to the mean and chunk padding is small.
    cnt = (np.bincount(row1 >> 7, minlength=GP)
           + np.bincount(row2 >> 7, minlength=GP))
    tile_rank = np.argsort(-cnt, kind="stable")
    assign = tile_rank.reshape(T, ncores)  # [slot, core] -> global tile
    core_of_tile = np.empty(GP, dtype=np.int64)
    slot_of_tile = np.empty(GP, dtype=np.int64)
    for s in range(T):
        for c in range(ncores):
            core_of_tile[assign[s, c]] = c
            slot_of_tile[assign[s, c]] = s

    pre = [
        _preprocess(row1, np.asarray(col1), np.asarray(val1),
                    n_nodes, core_of_tile, slot_of_tile, T, split, ncores),
        _preprocess(row2, np.asarray(col2), np.asarray(val2),
                    n_nodes, core_of_tile, slot_of_tile, T, split, ncores),
    ]
    caps = [pre[0][0], pre[1][0]]  # caps[h][t, half]

    # chunk layout per tile t: [half0: h1, h2][half1: h1, h2]
    half_chunks = caps[0] + caps[1]            # [T, 2]
    tile_chunks = half_chunks.sum(axis=1)      # [T]
    tile_off = np.concatenate([[0], np.cumsum(tile_chunks)])
    tot_chunks = int(tile_off[-1])
    pad_e = tot_chunks * P

    # chunk offset of (h, t, half) and edge offset
    coff = np.zeros((2, T, 2), dtype=np.int64)
    for t in range(T):
        base = tile_off[t]
        coff[0, t, 0] = base
        coff[1, t, 0] = base + caps[0][t, 0]
        coff[0, t, 1] = base + half_chunks[t, 0]
        coff[1, t, 1] = base + half_chunks[t, 0] + caps[0][t, 1]
    eoff = coff * P

    pad_idx = np.zeros((ncores, pad_e), dtype=np.int16)
    pad_ld = np.zeros((ncores, pad_e), dtype=np.float16)
    pad_val = np.zeros((ncores, pad_e), dtype=np.float16)

    for h in range(2):
        _, order, key_s, rank, idx, ld, v = pre[h]
        core_s = key_s // (T * 2)
        t_s = (key_s // 2) % T
        half_s = key_s % 2
        pos = eoff[h, t_s, half_s] + rank
        pad_idx[core_s, pos] = idx[order]
        pad_ld[core_s, pos] = ld[order].astype(np.float16)
        pad_val[core_s, pos] = np.asarray(v, dtype=np.float16)[order]

    # device layouts: dest/val as [128, tot_chunks, 2] fp16, each value
    # duplicated in pairs.  The selection-matrix build broadcasts these along
    # a 64-wide middle dim with an innermost stride-1 pair, which keeps the
    # vector engine in its 2-element/cycle mode (a plain stride-0 broadcast
    # operand halves DVE throughput).
    dest_arr = np.repeat(
        pad_ld.reshape(ncores, tot_chunks, P).transpose(0, 2, 1), 2, axis=2
    ).reshape(ncores, P, tot_chunks, 2)
    val_arr = np.repeat(
        pad_val.reshape(ncores, tot_chunks, P).transpose(0, 2, 1), 2, axis=2
    ).reshape(ncores, P, tot_chunks, 2)

    # idx: per (t, half) section, [16, n/16] wrap, replicated to 128 rows
    idx_cols = pad_e // 16
    idx_arr = np.zeros((ncores, 16, idx_cols), dtype=np.int16)
    sec_bounds = []
    for t in range(T):
        for half in range(2):
            o = int(eoff[0, t, half])
            n = int(half_chunks[t, half]) * P
            sec_bounds.append((o, n))
    for c in range(ncores):
        for o, n in sec_bounds:
            seg = pad_idx[c, o: o + n].reshape(n // 16, 16).T
            idx_arr[c, :, o // 16: (o + n) // 16] = seg
    idx_arr = np.tile(idx_arr, (1, 8, 1))  # [ncores, 128, idx_cols]

    maxsec = int(half_chunks.max())
    iota = np.ascontiguousarray(
        np.broadcast_to(
            np.arange(P, dtype=np.float16)[None, None, :], (P, maxsec, P)
        )
    ).reshape(P, maxsec, 64, 2)

    import ml_dtypes
    x8 = np.asarray(x, dtype=ml_dtypes.float8_e3m4)

    meta = dict(
        ncores=ncores, T=T, G=G, split=split, n_nodes=n_nodes,
        d_feat=d_feat, caps=caps, half_chunks=half_chunks,
        tile_off=tile_off, tot_chunks=tot_chunks, coff=coff, eoff=eoff,
        idx_cols=idx_cols, assign=assign,
        maxsec=maxsec,
    )
    per_core = [
        dict(x8=x8, idx=idx_arr[c], dest=dest_arr[c], val=val_arr[c],
             iota=iota)
        for c in range(ncores)
    ]
    return meta, per_core


def _build_program(meta, max_tiles=None):
    from concourse import bacc, mybir, tile

    T = meta["T"] if max_tiles is None else min(meta["T"], max_tiles)
    split = meta["split"]
    n_nodes = meta["n_nodes"]
    d = meta["d_feat"]
    caps = meta["caps"]
    half_chunks = meta["half_chunks"]
    coff = meta["coff"]
    eoff = meta["eoff"]
    tile_off = meta["tile_off"]
    tot_chunks = meta["tot_chunks"]
    idx_cols = meta["idx_cols"]

    maxsec = meta["maxsec"]

    nc = bacc.Bacc("TRN2", target_bir_lowering=False, debug=False,
                   num_devices=meta["ncores"], num_swdge_queues=NQ)

    fp16 = mybir.dt.float16
    f32 = mybir.dt.float32
    eq = mybir.AluOpType.is_equal
    mult = mybir.AluOpType.mult

    fp8 = mybir.dt.float8e3
    x8 = nc.dram_tensor("x8", [n_nodes, d], fp8, kind="ExternalInput")
    idx_d = nc.dram_tensor("idx", [P, idx_cols], mybir.dt.int16, kind="ExternalInput")
    dest_d = nc.dram_tensor("dest", [P, tot_chunks, 2], fp16, kind="ExternalInput")
    val_d = nc.dram_tensor("val", [P, tot_chunks, 2], fp16, kind="ExternalInput")
    iota_d = nc.dram_tensor("iota", [P, maxsec, 64, 2], fp16, kind="ExternalInput")
    out_d = nc.dram_tensor("out", [T * P, 2 * d], fp16, kind="ExternalOutput")

    with tile.TileContext(nc) as tc:
        with (
            tc.tile_pool(name="const", bufs=1) as constp,
            tc.tile_pool(name="msgs", bufs=8) as msgsp,
            tc.tile_pool(name="sel", bufs=4) as selp,
            tc.tile_pool(name="psum", bufs=4, space="PSUM") as psump,
            tc.tile_pool(name="stage", bufs=4) as stagep,
        ):
            iota_sb = constp.tile([P, maxsec, 64, 2], fp16, tag="iota")
            nc.sync.dma_start(iota_sb[:, :, :, :], iota_d[:, :, :, :])
            dest_sb = constp.tile([P, tot_chunks, 2], fp16, tag="dest")
            nc.sync.dma_start(dest_sb[:, :, :], dest_d[:, :, :])
            val_sb = constp.tile([P, tot_chunks, 2], fp16, tag="val")
            nc.sync.dma_start(val_sb[:, :, :], val_d[:, :, :])
            idx_sb = constp.tile([P, idx_cols], mybir.dt.int16, tag="idx")
            nc.sync.dma_start(idx_sb[:, :], idx_d[:, :])

            qrr = 0
            for t in range(T):
                ps = psump.tile([P, 2 * d], f32, tag="ps")
                first = True
                for half in range(2):
                    sec = int(half_chunks[t, half])
                    n = sec * P
                    o = int(eoff[0, t, half])
                    c0 = int(tile_off[t] + half * half_chunks[t, 0])
                    src = x8[0:split, :] if half == 0 else x8[split:n_nodes, :]
                    msgs = msgsp.tile([P, sec, d], fp8, tag="msgs")
                    nc.gpsimd.dma_gather(
                        msgs[:, :, :],
                        src,
                        idx_sb[:, o // 16: (o + n) // 16],
                        n,
                        n,
                        d,
                        single_packet=False,
                        queue_num=qrr % NQ,
                    )
                    qrr += 1
                    sel = selp.tile([P, sec, 64, 2], fp16, tag="sel")
                    dview = dest_sb[:, c0: c0 + sec, None, :].to_broadcast(
                        [P, sec, 64, 2])
                    vview = val_sb[:, c0: c0 + sec, None, :].to_broadcast(
                        [P, sec, 64, 2])
                    nc.vector.tensor_tensor(
                        out=sel[:, :, :, :], in0=iota_sb[:, :sec, :, :],
                        in1=dview, op=eq
                    )
                    nc.vector.tensor_tensor(
                        out=sel[:, :, :, :], in0=sel[:, :, :, :], in1=vview,
                        op=mult
                    )
                    for h in range(2):
                        nch = int(caps[h][t, half])
                        cl0 = int(coff[h, t, half]) - c0  # chunk offset in section
                        for j in range(nch):
                            last = (half == 1 and h == 1 and j == nch - 1)
                            nc.tensor.matmul(
                                ps[:, h * d: (h + 1) * d],
                                sel[:, cl0 + j, :, :],
                                msgs[:, cl0 + j, :],
                                start=first,
                                stop=last,
                            )
                            first = False
                st = stagep.tile([P, 2 * d], fp16, tag="st")
                nc.scalar.copy(st[:, :], ps[:, :])
                nc.sync.dma_start(
                    out_d[t * P: (t + 1) * P, :], st[:, :]
                )
    nc.compile()
    return nc


def _reassemble(meta, core_outs):
    """Scatter per-core [T*128, 512] slot outputs back to global rows."""
    n = meta["n_nodes"]
    d = meta["d_feat"]
    G = meta["G"]
    assign = meta["assign"]
    out = np.empty((n, 2 * d), dtype=np.float32)
    for c in range(meta["ncores"]):
        co = core_outs[c]
        for s in range(meta["T"]):
            g = int(assign[s, c])
            if g >= G:
                continue
            r0 = g * P
            nr = min(P, n - r0)
            out[r0: r0 + nr] = co[s * P: s * P + nr].astype(np.float32)
    return out


def kernel(x, row1, col1, val1, row2, col2, val2):
    from concourse.bass_utils import run_bass_kernel_spmd

    ncores = 8
    meta, per_core = _host_build(x, row1, col1, val1, row2, col2, val2, ncores)
    nc = _build_program(meta)
    res = run_bass_kernel_spmd(nc, per_core, list(range(ncores)))
    return _reassemble(meta, [res.results[c]["out"] for c in range(ncores)])


# revision 25
# speedup vs baseline: 94.3278x; 1.0017x over previous
"""H2GCNConv on 8 Trainium2 NeuronCores.

out = concat([A1 @ x, A2 @ x], axis=1) where A_h is sparse [N, N] given as
(row=dest, col=src, val) edge lists.

Strategy (dest-sharded SpMM via SWDGE gather + segment-matmul):
  - Destination-row tiles (128 rows) are distributed across the 8 cores
    with per-slot load balancing (slot s takes tiles ranked [8s, 8s+8) by
    edge count), and the host un-permutes the output rows at the end.
  - Host sorts each core's edges by (dest tile, column half, hop), pads
    each (tile, half, hop) section to whole 128-edge chunks.  The column
    half split keeps gather indices within int16 range.
  - x is cast to fp8 (e3m4, 4 mantissa bits) and replicated; each core
    runs ONE dma_gather per (tile, half) section (both hops' edges at
    once, 256B per edge) from HBM.  Gathers round-robin across 4 SWDGE
    queues so descriptor generation runs on all four Q7 core-pairs
    concurrently — the Q7 descriptor loop (~8ns/edge/pair) is the
    kernel's critical path; fp8 halves the DMA drain so it stays hidden.
  - Per (tile, half) section, a selection matrix S[e, c, d] = val[e,c] *
    (d == dest_local[e,c]) is built on the vector engine in two section-
    level passes (is_equal, then mult).  The broadcast operands (dest,
    val) are pair-duplicated on host and read with an innermost stride-1
    pair, which keeps the DVE in its 2-element/cycle mode (a plain
    stride-0 broadcast halves DVE throughput).
  - Per 128-edge chunk, the tensor engine computes
    psum[d, h*256:(h+1)*256] += S_chunk.T @ msgs_chunk as a mixed-dtype
    matmul (fp16 stationary sel x fp8e3 moving msgs -> fp32 PSUM), so
    edge weights keep fp16 precision while messages ride in fp8.  Both
    hops of a tile accumulate into a single [128, 512] PSUM bank
    (start=True zeroes the whole 2KB region).
  - PSUM is copied out through the scalar engine as fp16 and DMA'd to a
    fp16 output, which the host upcasts to fp32.  End-to-end relative
    error vs the fp64 oracle is ~1.4e-2 (dominated by fp8 message
    quantization), within the 2e-2 gate.
"""

import sys

if "/opt/trn_rl_repo" not in sys.path:
    sys.path.insert(0, "/opt/trn_rl_repo")

import numpy as np

P = 128
NQ = 4  # SWDGE queues


def _preprocess(rows, cols, vals, n_nodes, core_of_tile, slot_of_tile, T,
                split, ncores):
    """Sort one hop's edges by (core, slot, half); return per-(slot, half)
    capacities (in 128-edge chunks) and the sorted scatter metadata."""
    g = rows >> 7  # global dest tile
    core = core_of_tile[g]
    t = slot_of_tile[g]
    half = (cols >= split).astype(np.int64)
    idx = (cols - half * split).astype(np.int16)
    ld = rows & (P - 1)

    nsec = ncores * T * 2
    key = (core * T + t) * 2 + half
    counts = np.bincount(key, minlength=nsec).reshape(ncores, T, 2)
    caps = -(-counts.max(axis=0) // P)  # [T, 2] chunks
    caps = np.maximum(caps, 1)

    order = np.argsort(key, kind="stable")
    key_s = key[order]
    cs = np.concatenate([[0], np.cumsum(counts.reshape(-1))])
    rank = np.arange(len(rows)) - cs[key_s]
    return caps, order, key_s, rank, idx, ld, vals


def _host_build(x, row1, col1, val1, row2, col2, val2, ncores):
    n_nodes, d_feat = x.shape
    G = -(-n_nodes // P)       # global dest tiles
    T = -(-G // ncores)        # slots per core
    GP = T * ncores            # padded with dummy (empty) tiles
    split = -(-n_nodes // 2)
    assert split <= 32767 and n_nodes - split <= 32767

    row1 = np.asarray(row1)
    row2 = np.asarray(row2)
    # balance per-slot edge counts: slot s on each core takes one of the 8
    # tiles ranked [8s, 8s+8) by total edge count, so the per-slot cap
    # (max over cores) stays close to the mean and chunk padding is small.
    cnt = (np.bincount(row1 >> 7, minlength=GP)
           + np.bincount(row2 >> 7, minlength=GP))
    tile_rank = np.argsort(-cnt, kind="stable")
    assign = tile_rank.reshape(T, ncores)  # [slot, core] -> global tile
    core_of_tile = np.empty(GP, dtype=np.int64)
    slot_of_tile = np.empty(GP, dtype=np.int64)
    for s in range(T):
        for c in range(ncores):
            core_of_tile[assign[s, c]] = c
            slot_of_tile[assign[s, c]] = s

    pre = [
        _preprocess(row1, np.asarray(col1), np.asarray(val1),
                    n_nodes, core_of_tile, slot_of_tile, T, split, ncores),
        _preprocess(row2, np.asarray(col2), np.asarray(val2),
                    n_nodes, core_of_tile, slot_of_tile, T, split, ncores),
    ]
    caps = [pre[0][0], pre[1][0]]  # caps[h][t, half]

    # chunk layout per tile t: [half0: h1, h2][half1: h1, h2]
    half_chunks = caps[0] + caps[1]            # [T, 2]
    tile_chunks = half_chunks.sum(axis=1)      # [T]
    tile_off = np.concatenate([[0], np.cumsum(tile_chunks)])
    tot_chunks = int(tile_off[-1])
    pad_e = tot_chunks * P

    # chunk offset of (h, t, half) and edge offset
    coff = np.zeros((2, T, 2), dtype=np.int64)
    for t in range(T):
        base = tile_off[t]
        coff[0, t, 0] = base
        coff[1, t, 0] = base + caps[0][t, 0]
        coff[0, t, 1] = base + half_chunks[t, 0]
        coff[1, t, 1] = base + half_chunks[t, 0] + caps[0][t, 1]
    eoff = coff * P

    pad_idx = np.zeros((ncores, pad_e), dtype=np.int16)
    pad_ld = np.zeros((ncores, pad_e), dtype=np.float16)
    pad_val = np.zeros((ncores, pad_e), dtype=np.float16)

    for h in range(2):
        _, order, key_s, rank, idx, ld, v = pre[h]
        core_s = key_s // (T * 2)
        t_s = (key_s // 2) % T
        half_s = key_s % 2
        pos = eoff[h, t_s, half_s] + rank
        pad_idx[core_s, pos] = idx[order]
        pad_ld[core_s, pos] = ld[order].astype(np.float16)
        pad_val[core_s, pos] = np.asarray(v, dtype=np.float16)[order]

    # device layouts: dest/val as [128, tot_chunks, 2] fp16, each value
    # duplicated in pairs.  The selection-matrix build broadcasts these along
    # a 64-wide middle dim with an innermost stride-1 pair, which keeps the
    # vector engine in its 2-element/cycle mode (a plain stride-0 broadcast
    # operand halves DVE throughput).
    dest_arr = np.repeat(
        pad_ld.reshape(ncores, tot_chunks, P).transpose(0, 2, 1), 2, axis=2
    ).reshape(ncores, P, tot_chunks, 2)
    val_arr = np.repeat(
        pad_val.reshape(ncores, tot_chunks, P).transpose(0, 2, 1), 2, axis=2
    ).reshape(ncores, P, tot_chunks, 2)

    # idx: per (t, half) section, [16, n/16] wrap, replicated to 128 rows
    idx_cols = pad_e // 16
    idx_arr = np.zeros((ncores, 16, idx_cols), dtype=np.int16)
    sec_bounds = []
    for t in range(T):
        for half in range(2):
            o = int(eoff[0, t, half])
            n = int(half_chunks[t, half]) * P
            sec_bounds.append((o, n))
    for c in range(ncores):
        for o, n in sec_bounds:
            seg = pad_idx[c, o: o + n].reshape(n // 16, 16).T
            idx_arr[c, :, o // 16: (o + n) // 16] = seg
    idx_arr = np.tile(idx_arr, (1, 8, 1))  # [ncores, 128, idx_cols]

    maxsec = int(half_chunks.max())
    iota = np.ascontiguousarray(
        np.broadcast_to(
            np.arange(P, dtype=np.float16)[None, None, :], (P, maxsec, P)
        )
    ).reshape(P, maxsec, 64, 2)

    import ml_dtypes
    x8 = np.asarray(x, dtype=ml_dtypes.float8_e3m4)

    meta = dict(
        ncores=ncores, T=T, G=G, split=split, n_nodes=n_nodes,
        d_feat=d_feat, caps=caps, half_chunks=half_chunks,
        tile_off=tile_off, tot_chunks=tot_chunks, coff=coff, eoff=eoff,
        idx_cols=idx_cols, assign=assign,
        maxsec=maxsec,
    )
    per_core = [
        dict(x8=x8, idx=idx_arr[c], dest=dest_arr[c], val=val_arr[c],
             iota=iota)
        for c in range(ncores)
    ]
    return meta, per_core


def _build_program(meta, max_tiles=None):
    from concourse import bacc, mybir, tile

    T = meta["T"] if max_tiles is None else min(meta["T"], max_tiles)
    split = meta["split"]
    n_nodes = meta["n_nodes"]
    d = meta["d_feat"]
    caps = meta["caps"]
    half_chunks = meta["half_chunks"]
    coff = meta["coff"]
    eoff = meta["eoff"]
    tile_off = meta["tile_off"]
    tot_chunks = meta["tot_chunks"]
    idx_cols = meta["idx_cols"]

    maxsec = meta["maxsec"]

    nc = bacc.Bacc("TRN2", target_bir_lowering=False, debug=False,
                   num_devices=meta["ncores"], num_swdge_queues=NQ,
                   dynamic_dma_scratch_size=16384)

    fp16 = mybir.dt.float16
    f32 = mybir.dt.float32
    eq = mybir.AluOpType.is_equal
    mult = mybir.AluOpType.mult

    fp8 = mybir.dt.float8e3
    x8 = nc.dram_tensor("x8", [n_nodes, d], fp8, kind="ExternalInput")
    idx_d = nc.dram_tensor("idx", [P, idx_cols], mybir.dt.int16, kind="ExternalInput")
    dest_d = nc.dram_tensor("dest", [P, tot_chunks, 2], fp16, kind="ExternalInput")
    val_d = nc.dram_tensor("val", [P, tot_chunks, 2], fp16, kind="ExternalInput")
    iota_d = nc.dram_tensor("iota", [P, maxsec, 64, 2], fp16, kind="ExternalInput")
    out_d = nc.dram_tensor("out", [T * P, 2 * d], fp16, kind="ExternalOutput")

    with tile.TileContext(nc) as tc:
        with (
            tc.tile_pool(name="const", bufs=1) as constp,
            tc.tile_pool(name="msgs", bufs=10) as msgsp,
            tc.tile_pool(name="sel", bufs=6) as selp,
            tc.tile_pool(name="psum", bufs=6, space="PSUM") as psump,
            tc.tile_pool(name="stage", bufs=4) as stagep,
        ):
            iota_sb = constp.tile([P, maxsec, 64, 2], fp16, tag="iota")
            nc.sync.dma_start(iota_sb[:, :, :, :], iota_d[:, :, :, :])
            dest_sb = constp.tile([P, tot_chunks, 2], fp16, tag="dest")
            nc.sync.dma_start(dest_sb[:, :, :], dest_d[:, :, :])
            val_sb = constp.tile([P, tot_chunks, 2], fp16, tag="val")
            nc.sync.dma_start(val_sb[:, :, :], val_d[:, :, :])
            # split the idx load in four so the first gathers are not gated
            # on the full 5MB transfer
            idx_sb = constp.tile([P, idx_cols], mybir.dt.int16, tag="idx")
            qcol = [0]
            for k in range(4):
                # cut at a (tile, half) section boundary near k/4 of columns
                target = idx_cols * (k + 1) // 4
                cut = idx_cols
                for t in range(T):
                    for half in range(2):
                        b = int(eoff[0, t, half]) // 16
                        if b >= target:
                            cut = b
                            break
                    else:
                        continue
                    break
                cut = min(cut, idx_cols)
                if cut > qcol[-1]:
                    nc.sync.dma_start(
                        idx_sb[:, qcol[-1]: cut], idx_d[:, qcol[-1]: cut]
                    )
                    qcol.append(cut)
            if qcol[-1] < idx_cols:
                nc.sync.dma_start(
                    idx_sb[:, qcol[-1]:], idx_d[:, qcol[-1]:]
                )

            qrr = 0
            for t in range(T):
                ps = psump.tile([P, 2 * d], f32, tag="ps")
                first = True
                for half in range(2):
                    sec = int(half_chunks[t, half])
                    n = sec * P
                    o = int(eoff[0, t, half])
                    c0 = int(tile_off[t] + half * half_chunks[t, 0])
                    src = x8[0:split, :] if half == 0 else x8[split:n_nodes, :]
                    msgs = msgsp.tile([P, sec, d], fp8, tag="msgs")
                    nc.gpsimd.dma_gather(
                        msgs[:, :, :],
                        src,
                        idx_sb[:, o // 16: (o + n) // 16],
                        n,
                        n,
                        d,
                        single_packet=False,
                        queue_num=qrr % NQ,
                    )
                    qrr += 1
                    sel = selp.tile([P, sec, 64, 2], fp16, tag="sel")
                    dview = dest_sb[:, c0: c0 + sec, None, :].to_broadcast(
                        [P, sec, 64, 2])
                    vview = val_sb[:, c0: c0 + sec, None, :].to_broadcast(
                        [P, sec, 64, 2])
                    nc.vector.tensor_tensor(
                        out=sel[:, :, :, :], in0=iota_sb[:, :sec, :, :],
                        in1=dview, op=eq
                    )
                    nc.vector.tensor_tensor(
                        out=sel[:, :, :, :], in0=sel[:, :, :, :], in1=vview,
                        op=mult
                    )
                    for h in range(2):
                        nch = int(caps[h][t, half])
                        cl0 = int(coff[h, t, half]) - c0  # chunk offset in section
                        for j in range(nch):
                            last = (half == 1 and h == 1 and j == nch - 1)
                            nc.tensor.matmul(
                                ps[:, h * d: (h + 1) * d],
                                sel[:, cl0 + j, :, :],
                                msgs[:, cl0 + j, :],
                                start=first,
                                stop=last,
                            )
                            first = False
                st = stagep.tile([P, 2 * d], fp16, tag="st")
                nc.scalar.copy(st[:, :], ps[:, :])
                nc.sync.dma_start(
                    out_d[t * P: (t + 1) * P, :], st[:, :]
                )
    nc.compile()
    return nc


def _reassemble(meta, core_outs):
    """Scatter per-core [T*128, 512] slot outputs back to global rows."""
    n = meta["n_nodes"]
    d = meta["d_feat"]
    G = meta["G"]
    assign = meta["assign"]
    out = np.empty((n, 2 * d), dtype=np.float32)
    for c in range(meta["ncores"]):
        co = core_outs[c]
        for s in range(meta["T"]):
            g = int(assign[s, c])
            if g >= G:
                continue
            r0 = g * P
            nr = min(P, n - r0)
            out[r0: r0 + nr] = co[s * P: s * P + nr].astype(np.float32)
    return out


def kernel(x, row1, col1, val1, row2, col2, val2):
    from concourse.bass_utils import run_bass_kernel_spmd

    ncores = 8
    meta, per_core = _host_build(x, row1, col1, val1, row2, col2, val2, ncores)
    nc = _build_program(meta)
    res = run_bass_kernel_spmd(nc, per_core, list(range(ncores)))
    return _reassemble(meta, [res.results[c]["out"] for c in range(ncores)])


# revision 27
# speedup vs baseline: 95.3722x; 1.0111x over previous
"""H2GCNConv on 8 Trainium2 NeuronCores.

out = concat([A1 @ x, A2 @ x], axis=1) where A_h is sparse [N, N] given as
(row=dest, col=src, val) edge lists.

Strategy (dest-sharded SpMM via SWDGE gather + segment-matmul):
  - Destination-row tiles (128 rows) are distributed across the 8 cores
    with per-slot load balancing (slot s takes tiles ranked [8s, 8s+8) by
    edge count), and the host un-permutes the output rows at the end.
  - Host sorts each core's edges by (dest tile, column half, hop), pads
    each (tile, half, hop) section to whole 128-edge chunks.  The column
    half split keeps gather indices within int16 range.
  - x is cast to fp8 (e3m4, 4 mantissa bits) and replicated; each core
    runs ONE dma_gather per (tile, half) section (both hops' edges at
    once, 256B per edge) from HBM.  Gathers round-robin across 4 SWDGE
    queues so descriptor generation runs on all four Q7 core-pairs
    concurrently — the Q7 descriptor loop (~8ns/edge/pair) is the
    kernel's critical path; fp8 halves the DMA drain so it stays hidden.
  - Per (tile, half) section, a selection matrix S[e, c, d] = val[e,c] *
    (d == dest_local[e,c]) is built on the vector engine in two section-
    level passes (is_equal, then mult).  The broadcast operands (dest,
    val) are pair-duplicated on host and read with an innermost stride-1
    pair, which keeps the DVE in its 2-element/cycle mode (a plain
    stride-0 broadcast halves DVE throughput).
  - Per 128-edge chunk, the tensor engine computes
    psum[d, h*256:(h+1)*256] += S_chunk.T @ msgs_chunk as a mixed-dtype
    matmul (fp16 stationary sel x fp8e3 moving msgs -> fp32 PSUM), so
    edge weights keep fp16 precision while messages ride in fp8.  Both
    hops of a tile accumulate into a single [128, 512] PSUM bank
    (start=True zeroes the whole 2KB region).
  - PSUM is copied out through the scalar engine as fp16 and DMA'd to a
    fp16 output, which the host upcasts to fp32.  End-to-end relative
    error vs the fp64 oracle is ~1.4e-2 (dominated by fp8 message
    quantization), within the 2e-2 gate.
"""

import sys

if "/opt/trn_rl_repo" not in sys.path:
    sys.path.insert(0, "/opt/trn_rl_repo")

import numpy as np

P = 128
NQ = 4  # SWDGE queues


def _preprocess(rows, cols, vals, n_nodes, core_of_tile, slot_of_tile, T,
                split, ncores):
    """Sort one hop's edges by (core, slot, half); return per-(slot, half)
    capacities (in 128-edge chunks) and the sorted scatter metadata."""
    g = rows >> 7  # global dest tile
    core = core_of_tile[g]
    t = slot_of_tile[g]
    half = (cols >= split).astype(np.int64)
    idx = (cols - half * split).astype(np.int16)
    ld = rows & (P - 1)

    nsec = ncores * T * 2
    key = (core * T + t) * 2 + half
    counts = np.bincount(key, minlength=nsec).reshape(ncores, T, 2)
    caps = -(-counts.max(axis=0) // P)  # [T, 2] chunks
    caps = np.maximum(caps, 1)

    order = np.argsort(key, kind="stable")
    key_s = key[order]
    cs = np.concatenate([[0], np.cumsum(counts.reshape(-1))])
    rank = np.arange(len(rows)) - cs[key_s]
    return caps, order, key_s, rank, idx, ld, vals


def _host_build(x, row1, col1, val1, row2, col2, val2, ncores):
    n_nodes, d_feat = x.shape
    G = -(-n_nodes // P)       # global dest tiles
    T = -(-G // ncores)        # slots per core
    GP = T * ncores            # padded with dummy (empty) tiles
    split = -(-n_nodes // 2)
    assert split <= 32767 and n_nodes - split <= 32767

    row1 = np.asarray(row1)
    row2 = np.asarray(row2)
    # balance per-slot edge counts: slot s on each core takes one of the 8
    # tiles ranked [8s, 8s+8) by total edge count, so the per-slot cap
    # (max over cores) stays close to the mean and chunk padding is small.
    cnt = (np.bincount(row1 >> 7, minlength=GP)
           + np.bincount(row2 >> 7, minlength=GP))
    tile_rank = np.argsort(-cnt, kind="stable")
    assign = tile_rank.reshape(T, ncores)  # [slot, core] -> global tile
    core_of_tile = np.empty(GP, dtype=np.int64)
    slot_of_tile = np.empty(GP, dtype=np.int64)
    for s in range(T):
        for c in range(ncores):
            core_of_tile[assign[s, c]] = c
            slot_of_tile[assign[s, c]] = s

    pre = [
        _preprocess(row1, np.asarray(col1), np.asarray(val1),
                    n_nodes, core_of_tile, slot_of_tile, T, split, ncores),
        _preprocess(row2, np.asarray(col2), np.asarray(val2),
                    n_nodes, core_of_tile, slot_of_tile, T, split, ncores),
    ]
    caps = [pre[0][0], pre[1][0]]  # caps[h][t, half]

    # chunk layout per tile t: [half0: h1, h2][half1: h1, h2]
    half_chunks = caps[0] + caps[1]            # [T, 2]
    tile_chunks = half_chunks.sum(axis=1)      # [T]
    tile_off = np.concatenate([[0], np.cumsum(tile_chunks)])
    tot_chunks = int(tile_off[-1])
    pad_e = tot_chunks * P

    # chunk offset of (h, t, half) and edge offset
    coff = np.zeros((2, T, 2), dtype=np.int64)
    for t in range(T):
        base = tile_off[t]
        coff[0, t, 0] = base
        coff[1, t, 0] = base + caps[0][t, 0]
        coff[0, t, 1] = base + half_chunks[t, 0]
        coff[1, t, 1] = base + half_chunks[t, 0] + caps[0][t, 1]
    eoff = coff * P

    pad_idx = np.zeros((ncores, pad_e), dtype=np.int16)
    pad_ld = np.zeros((ncores, pad_e), dtype=np.float16)
    pad_val = np.zeros((ncores, pad_e), dtype=np.float16)

    for h in range(2):
        _, order, key_s, rank, idx, ld, v = pre[h]
        core_s = key_s // (T * 2)
        t_s = (key_s // 2) % T
        half_s = key_s % 2
        pos = eoff[h, t_s, half_s] + rank
        pad_idx[core_s, pos] = idx[order]
        pad_ld[core_s, pos] = ld[order].astype(np.float16)
        pad_val[core_s, pos] = np.asarray(v, dtype=np.float16)[order]

    # device layouts: dest/val as [128, tot_chunks, 2] fp16, each value
    # duplicated in pairs.  The selection-matrix build broadcasts these along
    # a 64-wide middle dim with an innermost stride-1 pair, which keeps the
    # vector engine in its 2-element/cycle mode (a plain stride-0 broadcast
    # operand halves DVE throughput).
    dest_arr = np.repeat(
        pad_ld.reshape(ncores, tot_chunks, P).transpose(0, 2, 1), 2, axis=2
    ).reshape(ncores, P, tot_chunks, 2)
    val_arr = np.repeat(
        pad_val.reshape(ncores, tot_chunks, P).transpose(0, 2, 1), 2, axis=2
    ).reshape(ncores, P, tot_chunks, 2)

    # idx: per (t, half) section, [16, n/16] wrap, replicated to 128 rows
    idx_cols = pad_e // 16
    idx_arr = np.zeros((ncores, 16, idx_cols), dtype=np.int16)
    sec_bounds = []
    for t in range(T):
        for half in range(2):
            o = int(eoff[0, t, half])
            n = int(half_chunks[t, half]) * P
            sec_bounds.append((o, n))
    for c in range(ncores):
        for o, n in sec_bounds:
            seg = pad_idx[c, o: o + n].reshape(n // 16, 16).T
            idx_arr[c, :, o // 16: (o + n) // 16] = seg
    idx_arr = np.tile(idx_arr, (1, 8, 1))  # [ncores, 128, idx_cols]

    maxsec = int(half_chunks.max())
    iota = np.ascontiguousarray(
        np.broadcast_to(
            np.arange(P, dtype=np.float16)[None, None, :], (P, maxsec, P)
        )
    ).reshape(P, maxsec, 64, 2)

    import ml_dtypes
    x8 = np.asarray(x, dtype=ml_dtypes.float8_e3m4)

    meta = dict(
        ncores=ncores, T=T, G=G, split=split, n_nodes=n_nodes,
        d_feat=d_feat, caps=caps, half_chunks=half_chunks,
        tile_off=tile_off, tot_chunks=tot_chunks, coff=coff, eoff=eoff,
        idx_cols=idx_cols, assign=assign,
        maxsec=maxsec,
    )
    per_core = [
        dict(x8=x8, idx=idx_arr[c], dest=dest_arr[c], val=val_arr[c],
             iota=iota)
        for c in range(ncores)
    ]
    return meta, per_core


def _build_program(meta, max_tiles=None):
    from concourse import bacc, mybir, tile

    T = meta["T"] if max_tiles is None else min(meta["T"], max_tiles)
    split = meta["split"]
    n_nodes = meta["n_nodes"]
    d = meta["d_feat"]
    caps = meta["caps"]
    half_chunks = meta["half_chunks"]
    coff = meta["coff"]
    eoff = meta["eoff"]
    tile_off = meta["tile_off"]
    tot_chunks = meta["tot_chunks"]
    idx_cols = meta["idx_cols"]

    maxsec = meta["maxsec"]

    nc = bacc.Bacc("TRN2", target_bir_lowering=False, debug=False,
                   num_devices=meta["ncores"], num_swdge_queues=NQ,
                   dynamic_dma_scratch_size=16384)

    fp16 = mybir.dt.float16
    f32 = mybir.dt.float32
    eq = mybir.AluOpType.is_equal
    mult = mybir.AluOpType.mult

    fp8 = mybir.dt.float8e3
    x8 = nc.dram_tensor("x8", [n_nodes, d], fp8, kind="ExternalInput")
    idx_d = nc.dram_tensor("idx", [P, idx_cols], mybir.dt.int16, kind="ExternalInput")
    dest_d = nc.dram_tensor("dest", [P, tot_chunks, 2], fp16, kind="ExternalInput")
    val_d = nc.dram_tensor("val", [P, tot_chunks, 2], fp16, kind="ExternalInput")
    iota_d = nc.dram_tensor("iota", [P, maxsec, 64, 2], fp16, kind="ExternalInput")
    out_d = nc.dram_tensor("out", [T * P, 2 * d], fp16, kind="ExternalOutput")

    with tile.TileContext(nc) as tc:
        with (
            tc.tile_pool(name="const", bufs=1) as constp,
            tc.tile_pool(name="msgs", bufs=10) as msgsp,
            tc.tile_pool(name="sel", bufs=6) as selp,
            tc.tile_pool(name="psum", bufs=6, space="PSUM") as psump,
            tc.tile_pool(name="stage", bufs=4) as stagep,
        ):
            iota_sb = constp.tile([P, maxsec, 64, 2], fp16, tag="iota")
            nc.sync.dma_start(iota_sb[:, :, :, :], iota_d[:, :, :, :])
            dest_sb = constp.tile([P, tot_chunks, 2], fp16, tag="dest")
            nc.sync.dma_start(dest_sb[:, :, :], dest_d[:, :, :])
            val_sb = constp.tile([P, tot_chunks, 2], fp16, tag="val")
            nc.sync.dma_start(val_sb[:, :, :], val_d[:, :, :])
            # split the idx load in four so the first gathers are not gated
            # on the full 5MB transfer
            idx_sb = constp.tile([P, idx_cols], mybir.dt.int16, tag="idx")
            qcol = [0]
            for k in range(4):
                # cut at a (tile, half) section boundary near k/4 of columns
                target = idx_cols * (k + 1) // 4
                cut = idx_cols
                for t in range(T):
                    for half in range(2):
                        b = int(eoff[0, t, half]) // 16
                        if b >= target:
                            cut = b
                            break
                    else:
                        continue
                    break
                cut = min(cut, idx_cols)
                if cut > qcol[-1]:
                    nc.sync.dma_start(
                        idx_sb[:, qcol[-1]: cut], idx_d[:, qcol[-1]: cut]
                    )
                    qcol.append(cut)
            if qcol[-1] < idx_cols:
                nc.sync.dma_start(
                    idx_sb[:, qcol[-1]:], idx_d[:, qcol[-1]:]
                )

            qrr = 0
            for t in range(T):
                ps = psump.tile([P, 2 * d], f32, tag="ps")
                first = True
                for half in range(2):
                    sec = int(half_chunks[t, half])
                    n = sec * P
                    o = int(eoff[0, t, half])
                    c0 = int(tile_off[t] + half * half_chunks[t, 0])
                    src = x8[0:split, :] if half == 0 else x8[split:n_nodes, :]
                    msgs = msgsp.tile([P, sec, d], fp8, tag="msgs")
                    nc.gpsimd.dma_gather(
                        msgs[:, :, :],
                        src,
                        idx_sb[:, o // 16: (o + n) // 16],
                        n,
                        n,
                        d,
                        single_packet=False,
                        queue_num=qrr % NQ,
                    )
                    qrr += 1
                    sel = selp.tile([P, sec, 64, 2], fp16, tag="sel")
                    dview = dest_sb[:, c0: c0 + sec, None, :].to_broadcast(
                        [P, sec, 64, 2])
                    vview = val_sb[:, c0: c0 + sec, None, :].to_broadcast(
                        [P, sec, 64, 2])
                    nc.vector.tensor_tensor(
                        out=sel[:, :, :, :], in0=iota_sb[:, :sec, :, :],
                        in1=dview, op=eq
                    )
                    nc.vector.tensor_tensor(
                        out=sel[:, :, :, :], in0=sel[:, :, :, :], in1=vview,
                        op=mult
                    )
                    for h in range(2):
                        nch = int(caps[h][t, half])
                        cl0 = int(coff[h, t, half]) - c0  # chunk offset in section
                        for j in range(nch):
                            last = (half == 1 and h == 1 and j == nch - 1)
                            nc.tensor.matmul(
                                ps[:, h * d: (h + 1) * d],
                                sel[:, cl0 + j, :, :],
                                msgs[:, cl0 + j, :],
                                start=first,
                                stop=last,
                            )
                            first = False
                st = stagep.tile([P, 2 * d], fp16, tag="st")
                nc.scalar.copy(st[:, :], ps[:, :])
                nc.sync.dma_start(
                    out_d[t * P: (t + 1) * P, :], st[:, :]
                )
    nc.compile()
    return nc


def _reassemble(meta, core_outs):
    """Scatter per-core [T*128, 512] slot outputs back to global rows."""
    n = meta["n_nodes"]
    d = meta["d_feat"]
    G = meta["G"]
    assign = meta["assign"]
    out = np.empty((n, 2 * d), dtype=np.float32)
    for c in range(meta["ncores"]):
        co = core_outs[c]
        for s in range(meta["T"]):
            g = int(assign[s, c])
            if g >= G:
                continue
            r0 = g * P
            nr = min(P, n - r0)
            out[r0: r0 + nr] = co[s * P: s * P + nr].astype(np.float32)
    return out


def kernel(x, row1, col1, val1, row2, col2, val2):
    from concourse.bass_utils import run_bass_kernel_spmd

    ncores = 8
    meta, per_core = _host_build(x, row1, col1, val1, row2, col2, val2, ncores)
    nc = _build_program(meta)
    res = run_bass_kernel_spmd(nc, per_core, list(range(ncores)))
    return _reassemble(meta, [res.results[c]["out"] for c in range(ncores)])


# revision 28
# speedup vs baseline: 95.7638x; 1.0041x over previous
"""H2GCNConv on 8 Trainium2 NeuronCores.

out = concat([A1 @ x, A2 @ x], axis=1) where A_h is sparse [N, N] given as
(row=dest, col=src, val) edge lists.

Strategy (dest-sharded SpMM via SWDGE gather + segment-matmul):
  - Destination-row tiles (128 rows) are distributed across the 8 cores
    with per-slot load balancing (slot s takes tiles ranked [8s, 8s+8) by
    edge count), and the host un-permutes the output rows at the end.
  - Host sorts each core's edges by (dest tile, column half, hop), pads
    each (tile, half, hop) section to whole 128-edge chunks.  The column
    half split keeps gather indices within int16 range.
  - x is cast to fp8 (e3m4, 4 mantissa bits) and replicated; each core
    runs ONE dma_gather per (tile, half) section (both hops' edges at
    once, 256B per edge) from HBM.  Gathers round-robin across 4 SWDGE
    queues so descriptor generation runs on all four Q7 core-pairs
    concurrently — the Q7 descriptor loop (~8ns/edge/pair) is the
    kernel's critical path; fp8 halves the DMA drain so it stays hidden.
  - Per (tile, half) section, a selection matrix S[e, c, d] = val[e,c] *
    (d == dest_local[e,c]) is built on the vector engine in two section-
    level passes (is_equal, then mult).  The broadcast operands (dest,
    val) are pair-duplicated on host and read with an innermost stride-1
    pair, which keeps the DVE in its 2-element/cycle mode (a plain
    stride-0 broadcast halves DVE throughput).
  - Per 128-edge chunk, the tensor engine computes
    psum[d, h*256:(h+1)*256] += S_chunk.T @ msgs_chunk as a mixed-dtype
    matmul (fp16 stationary sel x fp8e3 moving msgs -> fp32 PSUM), so
    edge weights keep fp16 precision while messages ride in fp8.  Both
    hops of a tile accumulate into a single [128, 512] PSUM bank
    (start=True zeroes the whole 2KB region).
  - PSUM is copied out through the scalar engine as fp16 and DMA'd to a
    fp16 output, which the host upcasts to fp32.  End-to-end relative
    error vs the fp64 oracle is ~1.4e-2 (dominated by fp8 message
    quantization), within the 2e-2 gate.
"""

import sys

if "/opt/trn_rl_repo" not in sys.path:
    sys.path.insert(0, "/opt/trn_rl_repo")

import numpy as np

P = 128
NQ = 4  # SWDGE queues


def _preprocess(rows, cols, vals, n_nodes, core_of_tile, slot_of_tile, T,
                split, ncores):
    """Sort one hop's edges by (core, slot, half); return per-(slot, half)
    capacities (in 128-edge chunks) and the sorted scatter metadata."""
    g = rows >> 7  # global dest tile
    core = core_of_tile[g]
    t = slot_of_tile[g]
    half = (cols >= split).astype(np.int64)
    idx = (cols - half * split).astype(np.int16)
    ld = rows & (P - 1)

    nsec = ncores * T * 2
    key = (core * T + t) * 2 + half
    counts = np.bincount(key, minlength=nsec).reshape(ncores, T, 2)
    caps = -(-counts.max(axis=0) // P)  # [T, 2] chunks
    caps = np.maximum(caps, 1)

    order = np.argsort(key, kind="stable")
    key_s = key[order]
    cs = np.concatenate([[0], np.cumsum(counts.reshape(-1))])
    rank = np.arange(len(rows)) - cs[key_s]
    return caps, order, key_s, rank, idx, ld, vals


def _host_build(x, row1, col1, val1, row2, col2, val2, ncores):
    n_nodes, d_feat = x.shape
    G = -(-n_nodes // P)       # global dest tiles
    T = -(-G // ncores)        # slots per core
    GP = T * ncores            # padded with dummy (empty) tiles
    split = -(-n_nodes // 2)
    assert split <= 32767 and n_nodes - split <= 32767

    row1 = np.asarray(row1)
    row2 = np.asarray(row2)
    # balance per-slot edge counts: slot s on each core takes one of the 8
    # tiles ranked [8s, 8s+8) by total edge count, so the per-slot cap
    # (max over cores) stays close to the mean and chunk padding is small.
    cnt = (np.bincount(row1 >> 7, minlength=GP)
           + np.bincount(row2 >> 7, minlength=GP))
    tile_rank = np.argsort(-cnt, kind="stable")
    assign = tile_rank.reshape(T, ncores)  # [slot, core] -> global tile
    core_of_tile = np.empty(GP, dtype=np.int64)
    slot_of_tile = np.empty(GP, dtype=np.int64)
    for s in range(T):
        for c in range(ncores):
            core_of_tile[assign[s, c]] = c
            slot_of_tile[assign[s, c]] = s

    pre = [
        _preprocess(row1, np.asarray(col1), np.asarray(val1),
                    n_nodes, core_of_tile, slot_of_tile, T, split, ncores),
        _preprocess(row2, np.asarray(col2), np.asarray(val2),
                    n_nodes, core_of_tile, slot_of_tile, T, split, ncores),
    ]
    caps = [pre[0][0], pre[1][0]]  # caps[h][t, half]

    # chunk layout per tile t: [half0: h1, h2][half1: h1, h2]
    half_chunks = caps[0] + caps[1]            # [T, 2]
    tile_chunks = half_chunks.sum(axis=1)      # [T]
    tile_off = np.concatenate([[0], np.cumsum(tile_chunks)])
    tot_chunks = int(tile_off[-1])
    pad_e = tot_chunks * P

    # chunk offset of (h, t, half) and edge offset
    coff = np.zeros((2, T, 2), dtype=np.int64)
    for t in range(T):
        base = tile_off[t]
        coff[0, t, 0] = base
        coff[1, t, 0] = base + caps[0][t, 0]
        coff[0, t, 1] = base + half_chunks[t, 0]
        coff[1, t, 1] = base + half_chunks[t, 0] + caps[0][t, 1]
    eoff = coff * P

    pad_idx = np.zeros((ncores, pad_e), dtype=np.int16)
    pad_ld = np.zeros((ncores, pad_e), dtype=np.float16)
    pad_val = np.zeros((ncores, pad_e), dtype=np.float16)

    for h in range(2):
        _, order, key_s, rank, idx, ld, v = pre[h]
        core_s = key_s // (T * 2)
        t_s = (key_s // 2) % T
        half_s = key_s % 2
        pos = eoff[h, t_s, half_s] + rank
        pad_idx[core_s, pos] = idx[order]
        pad_ld[core_s, pos] = ld[order].astype(np.float16)
        pad_val[core_s, pos] = np.asarray(v, dtype=np.float16)[order]

    # device layouts: dest/val as [128, tot_chunks, 2] fp16, each value
    # duplicated in pairs.  The selection-matrix build broadcasts these along
    # a 64-wide middle dim with an innermost stride-1 pair, which keeps the
    # vector engine in its 2-element/cycle mode (a plain stride-0 broadcast
    # operand halves DVE throughput).
    dest_arr = np.repeat(
        pad_ld.reshape(ncores, tot_chunks, P).transpose(0, 2, 1), 2, axis=2
    ).reshape(ncores, P, tot_chunks, 2)
    val_arr = np.repeat(
        pad_val.reshape(ncores, tot_chunks, P).transpose(0, 2, 1), 2, axis=2
    ).reshape(ncores, P, tot_chunks, 2)

    # idx: per (t, half) section, [16, n/16] wrap, replicated to 128 rows
    idx_cols = pad_e // 16
    idx_arr = np.zeros((ncores, 16, idx_cols), dtype=np.int16)
    sec_bounds = []
    for t in range(T):
        for half in range(2):
            o = int(eoff[0, t, half])
            n = int(half_chunks[t, half]) * P
            sec_bounds.append((o, n))
    for c in range(ncores):
        for o, n in sec_bounds:
            seg = pad_idx[c, o: o + n].reshape(n // 16, 16).T
            idx_arr[c, :, o // 16: (o + n) // 16] = seg
    idx_arr = np.tile(idx_arr, (1, 8, 1))  # [ncores, 128, idx_cols]

    maxsec = int(half_chunks.max())
    iota = np.ascontiguousarray(
        np.broadcast_to(
            np.arange(P, dtype=np.float16)[None, None, :], (P, maxsec, P)
        )
    ).reshape(P, maxsec, 64, 2)

    import ml_dtypes
    x8 = np.asarray(x, dtype=ml_dtypes.float8_e3m4)

    meta = dict(
        ncores=ncores, T=T, G=G, split=split, n_nodes=n_nodes,
        d_feat=d_feat, caps=caps, half_chunks=half_chunks,
        tile_off=tile_off, tot_chunks=tot_chunks, coff=coff, eoff=eoff,
        idx_cols=idx_cols, assign=assign,
        maxsec=maxsec,
    )
    per_core = [
        dict(x8=x8, idx=idx_arr[c], dest=dest_arr[c], val=val_arr[c],
             iota=iota)
        for c in range(ncores)
    ]
    return meta, per_core


def _build_program(meta, max_tiles=None):
    from concourse import bacc, mybir, tile

    T = meta["T"] if max_tiles is None else min(meta["T"], max_tiles)
    split = meta["split"]
    n_nodes = meta["n_nodes"]
    d = meta["d_feat"]
    caps = meta["caps"]
    half_chunks = meta["half_chunks"]
    coff = meta["coff"]
    eoff = meta["eoff"]
    tile_off = meta["tile_off"]
    tot_chunks = meta["tot_chunks"]
    idx_cols = meta["idx_cols"]

    maxsec = meta["maxsec"]

    nc = bacc.Bacc("TRN2", target_bir_lowering=False, debug=False,
                   num_devices=meta["ncores"], num_swdge_queues=NQ,
                   dynamic_dma_scratch_size=16384)

    fp16 = mybir.dt.float16
    f32 = mybir.dt.float32
    eq = mybir.AluOpType.is_equal
    mult = mybir.AluOpType.mult

    fp8 = mybir.dt.float8e3
    x8 = nc.dram_tensor("x8", [n_nodes, d], fp8, kind="ExternalInput")
    idx_d = nc.dram_tensor("idx", [P, idx_cols], mybir.dt.int16, kind="ExternalInput")
    dest_d = nc.dram_tensor("dest", [P, tot_chunks, 2], fp16, kind="ExternalInput")
    val_d = nc.dram_tensor("val", [P, tot_chunks, 2], fp16, kind="ExternalInput")
    iota_d = nc.dram_tensor("iota", [P, maxsec, 64, 2], fp16, kind="ExternalInput")
    out_d = nc.dram_tensor("out", [T * P, 2 * d], fp16, kind="ExternalOutput")

    with tile.TileContext(nc) as tc:
        with (
            tc.tile_pool(name="const", bufs=1) as constp,
            tc.tile_pool(name="msgs", bufs=10) as msgsp,
            tc.tile_pool(name="sel", bufs=6) as selp,
            tc.tile_pool(name="psum", bufs=6, space="PSUM") as psump,
            tc.tile_pool(name="stage", bufs=4) as stagep,
        ):
            # idx first, leading with a small slice covering the first three
            # tiles, so the first gathers are not gated on the full 5MB load
            # (or on the other constants, which only the later DVE/PE work
            # needs)
            idx_sb = constp.tile([P, idx_cols], mybir.dt.int16, tag="idx")
            cuts = [0]
            for tcut in (3, 12, 24, 36):
                if tcut < T:
                    b = int(eoff[0, tcut, 0]) // 16
                    if b > cuts[-1]:
                        cuts.append(b)
            cuts.append(idx_cols)
            nc.sync.dma_start(idx_sb[:, : cuts[1]], idx_d[:, : cuts[1]])
            iota_sb = constp.tile([P, maxsec, 64, 2], fp16, tag="iota")
            nc.sync.dma_start(iota_sb[:, :, :, :], iota_d[:, :, :, :])
            dest_sb = constp.tile([P, tot_chunks, 2], fp16, tag="dest")
            nc.sync.dma_start(dest_sb[:, :, :], dest_d[:, :, :])
            val_sb = constp.tile([P, tot_chunks, 2], fp16, tag="val")
            nc.sync.dma_start(val_sb[:, :, :], val_d[:, :, :])
            for a, b in zip(cuts[1:], cuts[2:]):
                nc.sync.dma_start(idx_sb[:, a:b], idx_d[:, a:b])

            qrr = 0
            for t in range(T):
                ps = psump.tile([P, 2 * d], f32, tag="ps")
                first = True
                for half in range(2):
                    sec = int(half_chunks[t, half])
                    n = sec * P
                    o = int(eoff[0, t, half])
                    c0 = int(tile_off[t] + half * half_chunks[t, 0])
                    src = x8[0:split, :] if half == 0 else x8[split:n_nodes, :]
                    msgs = msgsp.tile([P, sec, d], fp8, tag="msgs")
                    nc.gpsimd.dma_gather(
                        msgs[:, :, :],
                        src,
                        idx_sb[:, o // 16: (o + n) // 16],
                        n,
                        n,
                        d,
                        single_packet=False,
                        queue_num=qrr % NQ,
                    )
                    qrr += 1
                    sel = selp.tile([P, sec, 64, 2], fp16, tag="sel")
                    dview = dest_sb[:, c0: c0 + sec, None, :].to_broadcast(
                        [P, sec, 64, 2])
                    vview = val_sb[:, c0: c0 + sec, None, :].to_broadcast(
                        [P, sec, 64, 2])
                    nc.vector.tensor_tensor(
                        out=sel[:, :, :, :], in0=iota_sb[:, :sec, :, :],
                        in1=dview, op=eq
                    )
                    nc.vector.tensor_tensor(
                        out=sel[:, :, :, :], in0=sel[:, :, :, :], in1=vview,
                        op=mult
                    )
                    for h in range(2):
                        nch = int(caps[h][t, half])
                        cl0 = int(coff[h, t, half]) - c0  # chunk offset in section
                        for j in range(nch):
                            last = (half == 1 and h == 1 and j == nch - 1)
                            nc.tensor.matmul(
                                ps[:, h * d: (h + 1) * d],
                                sel[:, cl0 + j, :, :],
                                msgs[:, cl0 + j, :],
                                start=first,
                                stop=last,
                            )
                            first = False
                st = stagep.tile([P, 2 * d], fp16, tag="st")
                nc.scalar.copy(st[:, :], ps[:, :])
                nc.sync.dma_start(
                    out_d[t * P: (t + 1) * P, :], st[:, :]
                )
    nc.compile()
    return nc


def _reassemble(meta, core_outs):
    """Scatter per-core [T*128, 512] slot outputs back to global rows."""
    n = meta["n_nodes"]
    d = meta["d_feat"]
    G = meta["G"]
    assign = meta["assign"]
    out = np.empty((n, 2 * d), dtype=np.float32)
    for c in range(meta["ncores"]):
        co = core_outs[c]
        for s in range(meta["T"]):
            g = int(assign[s, c])
            if g >= G:
                continue
            r0 = g * P
            nr = min(P, n - r0)
            out[r0: r0 + nr] = co[s * P: s * P + nr].astype(np.float32)
    return out


def kernel(x, row1, col1, val1, row2, col2, val2):
    from concourse.bass_utils import run_bass_kernel_spmd

    ncores = 8
    meta, per_core = _host_build(x, row1, col1, val1, row2, col2, val2, ncores)
    nc = _build_program(meta)
    res = run_bass_kernel_spmd(nc, per_core, list(range(ncores)))
    return _reassemble(meta, [res.results[c]["out"] for c in range(ncores)])
